# revision 13
# baseline (speedup 1.0000x reference)
"""Trainium2 SPMD kernel for AdaptiveMultimodalFusion (8 NeuronCores, data parallel).

Math notes (exact simplifications of the reference):
  - Each _mha_seq1 has seq_len 1, so softmax over the single key is exactly 1.0
    and the MHA output is (x_kv @ wv + bv) @ wo + bo -- independent of x_q and
    of the q/k projections.
  - Chained MHAs therefore collapse to the LAST one in each chain:
        attended_image    = f(proj_clinical; attn[image_clinical])
        attended_text     = f(proj_clinical; attn[text_clinical])
        attended_clinical = f(proj_text;     attn[clinical_text])
  - Everything up to the first LayerNorm is affine in the features, so it folds
    (on host, in float64) into two matrices + one bias:
        z = feat_clinical @ Mc + feat_text @ Mt + zb          [B, 1024]
    with h = gelu(LN(z) * g1 + be1), fused = LN(h @ W2 + b2) * g2 + be2.
  - Uncertainty heads cannot fold (relu/softplus): per modality
        u_m = mean(softplus(relu(feat_m @ w1 + b1) @ w2 + b2))
    Each core reduces its batch shard to a partial sum; host finishes the mean.

Sharding: batch 16384 split 8 ways (2048 rows/core), all parameters replicated.
Features are transposed on host to [D, Bs] so the contraction dim lands on SBUF
partitions. Matmuls run in bf16 with fp32 PSUM accumulation; LayerNorm,
activations, and reductions run in fp32.
"""

import os
import sys
import types

import numpy as np
import ml_dtypes

BF16 = ml_dtypes.bfloat16

N_CORES = 8
B = 16384
BS = B // N_CORES  # batch rows per core
D_IMG, D_TXT, D_CLI = 2048, 768, 256
FUSION = 512
NF1 = 2 * FUSION  # 1024, first fusion layer width
NCAT = 3 * FUSION
CH = 512  # batch columns per chunk on device
NCH = BS // CH  # chunks per core
LN_EPS = 1e-5

MODS = ["image", "text", "clinical"]


def _build(flags):
    """Build the per-core Bass graph. flags: dict of triviality flags."""
    import concourse.bass as bass
    import concourse.mybir as mybir
    import concourse.tile as tile
    from concourse import bacc
    from concourse.masks import make_identity
    from contextlib import ExitStack

    BF = mybir.dt.bfloat16
    F32 = mybir.dt.float32
    AF = mybir.ActivationFunctionType
    ALU = mybir.AluOpType

    nc = bacc.Bacc("TRN2", target_bir_lowering=False, debug=False,
                   num_devices=N_CORES)

    # ---- DRAM I/O ----
    fiT = nc.dram_tensor("fiT", [D_IMG, BS], BF, kind="ExternalInput")
    ftT = nc.dram_tensor("ftT", [D_TXT, BS], BF, kind="ExternalInput")
    fcT = nc.dram_tensor("fcT", [D_CLI, BS], BF, kind="ExternalInput")
    Mc_d = nc.dram_tensor("Mc", [D_CLI, NF1], BF, kind="ExternalInput")
    Mt_d = nc.dram_tensor("Mt", [D_TXT, NF1], BF, kind="ExternalInput")
    W2_d = nc.dram_tensor("W2", [NF1, FUSION], BF, kind="ExternalInput")
    w1i_d = nc.dram_tensor("w1i", [D_IMG, 128], BF, kind="ExternalInput")
    w1t_d = nc.dram_tensor("w1t", [D_TXT, 128], BF, kind="ExternalInput")
    w1c_d = nc.dram_tensor("w1c", [D_CLI, 128], BF, kind="ExternalInput")
    w2u_d = nc.dram_tensor("w2u", [128, 3], BF, kind="ExternalInput")
    b1u_d = nc.dram_tensor("b1u", [128, 3], F32, kind="ExternalInput")
    b2u_d = nc.dram_tensor("b2u", [3, 1], F32, kind="ExternalInput")
    # General-path parameters (used only when the fast-path flags are off).
    zb_d = nc.dram_tensor("zb", [1, NF1], BF, kind="ExternalInput")
    b2f_d = nc.dram_tensor("b2f", [1, FUSION], BF, kind="ExternalInput")
    g1_d = nc.dram_tensor("g1", [1, NF1], F32, kind="ExternalInput")
    be1_d = nc.dram_tensor("be1", [1, NF1], F32, kind="ExternalInput")
    g2_d = nc.dram_tensor("g2", [1, FUSION], F32, kind="ExternalInput")
    be2_d = nc.dram_tensor("be2", [1, FUSION], F32, kind="ExternalInput")

    out_f = nc.dram_tensor("out_fused", [BS, FUSION], F32, kind="ExternalOutput")
    out_u = nc.dram_tensor("out_unc", [3, 1], F32, kind="ExternalOutput")

    KI, KT, KC = D_IMG // 128, D_TXT // 128, D_CLI // 128  # 16, 6, 2
    KW2 = NF1 // 128  # 8

    with tile.TileContext(nc) as tc, ExitStack() as ctx:
        wpool = ctx.enter_context(tc.tile_pool(name="weights", bufs=1))
        fpool = ctx.enter_context(tc.tile_pool(name="feats", bufs=2))
        tpool = ctx.enter_context(tc.tile_pool(name="temps", bufs=3))
        spool = ctx.enter_context(tc.tile_pool(name="stats", bufs=6))
        ps_mm = ctx.enter_context(tc.tile_pool(name="ps_mm", bufs=5, space="PSUM"))
        ps_tr = ctx.enter_context(tc.tile_pool(name="ps_tr", bufs=3, space="PSUM"))

        # ---- constant / weight tiles ----
        Mc_sb = wpool.tile([128, KC, NF1], BF, tag="Mc")
        nc.sync.dma_start(out=Mc_sb, in_=Mc_d.rearrange("(t p) n -> p t n", p=128))
        Mt_sb = wpool.tile([128, KT, NF1], BF, tag="Mt")
        nc.sync.dma_start(out=Mt_sb, in_=Mt_d.rearrange("(t p) n -> p t n", p=128))
        W2_sb = wpool.tile([128, KW2, FUSION], BF, tag="W2")
        nc.sync.dma_start(out=W2_sb, in_=W2_d.rearrange("(t p) n -> p t n", p=128))
        w1i_sb = wpool.tile([128, KI, 128], BF, tag="w1i")
        nc.sync.dma_start(out=w1i_sb, in_=w1i_d.rearrange("(t p) n -> p t n", p=128))
        w1t_sb = wpool.tile([128, KT, 128], BF, tag="w1t")
        nc.sync.dma_start(out=w1t_sb, in_=w1t_d.rearrange("(t p) n -> p t n", p=128))
        w1c_sb = wpool.tile([128, KC, 128], BF, tag="w1c")
        nc.sync.dma_start(out=w1c_sb, in_=w1c_d.rearrange("(t p) n -> p t n", p=128))
        w2u_sb = wpool.tile([128, 3], BF, tag="w2u")
        nc.sync.dma_start(out=w2u_sb, in_=w2u_d[:, :])
        b1u_sb = wpool.tile([128, 3], F32, tag="b1u")
        nc.sync.dma_start(out=b1u_sb, in_=b1u_d[:, :])
        b2u_sb = wpool.tile([3, 1], F32, tag="b2u")
        nc.sync.dma_start(out=b2u_sb, in_=b2u_d[:, :])

        ident = wpool.tile([128, 128], BF, tag="ident")
        make_identity(nc, ident)
        # magic constant for the bit-trick rsqrt seed (int32 view)
        magic_sb = wpool.tile([128, 1], mybir.dt.int32, tag="magic")
        nc.vector.memset(magic_sb, 0x5F375A86)
        acc_sb = wpool.tile([3, 1], F32, tag="acc")
        # softplus staging: partition = modality, free = (chunk, batch col)
        sp_all = wpool.tile([3, NCH * CH], F32, tag="sp_all")

        need_ones = (not flags["zb_triv"]) or (not flags["b2f_triv"])
        if need_ones:
            ones_sb = wpool.tile([1, 128], BF, tag="ones")
            nc.vector.memset(ones_sb, 1.0)
        if not flags["zb_triv"]:
            zb_sb = wpool.tile([1, NF1], BF, tag="zb")
            nc.sync.dma_start(out=zb_sb, in_=zb_d[:, :])
        if not flags["b2f_triv"]:
            b2f_sb = wpool.tile([1, FUSION], BF, tag="b2f")
            nc.sync.dma_start(out=b2f_sb, in_=b2f_d[:, :])
        if not flags["g1_triv"]:
            g1_sb = wpool.tile([128, NF1], F32, tag="g1")
            nc.sync.dma_start(out=g1_sb, in_=g1_d.to_broadcast([128, NF1]))
            be1_sb = wpool.tile([128, NF1], F32, tag="be1")
            nc.sync.dma_start(out=be1_sb, in_=be1_d.to_broadcast([128, NF1]))
        if not flags["g2_triv"]:
            g2_sb = wpool.tile([128, FUSION], F32, tag="g2")
            nc.sync.dma_start(out=g2_sb, in_=g2_d.to_broadcast([128, FUSION]))
            be2_sb = wpool.tile([128, FUSION], F32, tag="be2")
            nc.sync.dma_start(out=be2_sb, in_=be2_d.to_broadcast([128, FUSION]))

        fiT_r = fiT.rearrange("(t p) b -> p t b", p=128)
        ftT_r = ftT.rearrange("(t p) b -> p t b", p=128)
        fcT_r = fcT.rearrange("(t p) b -> p t b", p=128)

        def rsqrt_dve(var_col):
            """rstd = 1/sqrt(var + eps) on the DVE only (no ACT table):
            bit-trick seed + one Newton-Raphson step (~4e-6 rel err)."""
            v = spool.tile([128, 1], mybir.dt.float32, tag="rs_v")
            nc.vector.tensor_scalar(out=v, in0=var_col, scalar1=LN_EPS,
                                    scalar2=None, op0=ALU.add)
            yi = spool.tile([128, 1], mybir.dt.int32, tag="rs_i")
            nc.vector.tensor_scalar(out=yi, in0=v.bitcast(mybir.dt.int32),
                                    scalar1=1, scalar2=None,
                                    op0=ALU.logical_shift_right)
            nc.vector.tensor_tensor(out=yi, in0=magic_sb, in1=yi,
                                    op=ALU.subtract)
            y0 = yi.bitcast(mybir.dt.float32)
            t = spool.tile([128, 1], mybir.dt.float32, tag="rs_t")
            nc.vector.tensor_tensor(out=t, in0=v, in1=y0, op=ALU.mult)
            nc.vector.tensor_tensor(out=t, in0=t, in1=y0, op=ALU.mult)
            nc.vector.tensor_scalar(out=t, in0=t, scalar1=-0.5, scalar2=1.5,
                                    op0=ALU.mult, op1=ALU.add)
            rstd = spool.tile([128, 1], mybir.dt.float32, tag="rs_y")
            nc.vector.tensor_tensor(out=rstd, in0=y0, in1=t, op=ALU.mult)
            return rstd

        def layernorm_apply(z_parts, t_out, width, g_triv, g_sb_, be_sb_):
            """z_parts: list of PSUM tiles covering `width` columns.
            Writes normalized (and optionally affine-transformed) result to
            t_out (SBUF)."""
            nsub = len(z_parts)
            stat = spool.tile([128, nsub, 6], mybir.dt.float32, tag="stat")
            for s, zp in enumerate(z_parts):
                nc.vector.bn_stats(stat[:, s, :], zp)
            mv = spool.tile([128, 2], mybir.dt.float32, tag="mv")
            nc.vector.bn_aggr(mv, stat)
            rstd = rsqrt_dve(mv[:, 1:2])
            sub = width // nsub
            for s, zp in enumerate(z_parts):
                dst = t_out[:, s * sub:(s + 1) * sub]
                if g_triv:
                    nc.vector.tensor_scalar(
                        out=dst, in0=zp, scalar1=mv[:, 0:1], scalar2=rstd,
                        op0=ALU.subtract, op1=ALU.mult)
                else:
                    tmp = spool.tile([128, sub], mybir.dt.float32, tag="lntmp")
                    nc.vector.tensor_scalar(
                        out=tmp, in0=zp, scalar1=mv[:, 0:1], scalar2=rstd,
                        op0=ALU.subtract, op1=ALU.mult)
                    nc.vector.tensor_mul(tmp, tmp, g_sb_[:, s * sub:(s + 1) * sub])
                    nc.vector.tensor_add(dst, tmp, be_sb_[:, s * sub:(s + 1) * sub])

        def emit_z(fc_sb, ft_sb, i):
            """z matmuls for batch tile i of the current chunk. z0 (cols
            0:512) accumulates first so LN stats can start early."""
            isl = slice(i * 128, (i + 1) * 128)
            z0 = ps_mm.tile([128, FUSION], mybir.dt.float32, tag="mm")
            z1 = ps_mm.tile([128, FUSION], mybir.dt.float32, tag="mm")
            ksrc = ([(fc_sb, k, Mc_sb, k) for k in range(KC)]
                    + [(ft_sb, k, Mt_sb, k) for k in range(KT)])
            nz = len(ksrc) + (0 if flags["zb_triv"] else 1)
            for half, zp in ((0, z0), (1, z1)):
                for ki, (fsb, kk, msb, mk) in enumerate(ksrc):
                    nc.tensor.matmul(zp, fsb[:, kk, isl],
                                     msb[:, mk, half * FUSION:(half + 1) * FUSION],
                                     start=(ki == 0), stop=(ki == nz - 1))
                if not flags["zb_triv"]:
                    nc.tensor.matmul(zp, ones_sb,
                                     zb_sb[:, half * FUSION:(half + 1) * FUSION],
                                     start=False, stop=True)
            return z0, z1

        def emit_ln1(z0, z1):
            t_sb = tpool.tile([128, NF1], BF, tag="t")
            layernorm_apply([z0, z1], t_sb, NF1, flags["g1_triv"],
                            None if flags["g1_triv"] else g1_sb,
                            None if flags["g1_triv"] else be1_sb)
            return t_sb

        def emit_tr(t_sb):
            tr_ps = ps_tr.tile([128, KW2, 128], BF, tag="aux")
            for j in range(KW2):
                nc.tensor.transpose(tr_ps[:, j, :],
                                    t_sb[:, j * 128:(j + 1) * 128], ident)
            return tr_ps

        def emit_gelu(tr_ps):
            hT = tpool.tile([128, KW2, 128], BF, tag="hT")
            nc.scalar.activation(hT, tr_ps, func=AF.Gelu)
            return hT

        def emit_y_out(hT, row0):
            yp = ps_mm.tile([128, FUSION], mybir.dt.float32, tag="mm")
            ny = KW2 + (0 if flags["b2f_triv"] else 1)
            for j in range(KW2):
                nc.tensor.matmul(yp, hT[:, j, :], W2_sb[:, j, :],
                                 start=(j == 0), stop=(j == ny - 1))
            if not flags["b2f_triv"]:
                nc.tensor.matmul(yp, ones_sb, b2f_sb, start=False, stop=True)
            o_sb = tpool.tile([128, FUSION], mybir.dt.float32, tag="o")
            layernorm_apply([yp], o_sb, FUSION, flags["g2_triv"],
                            None if flags["g2_triv"] else g2_sb,
                            None if flags["g2_triv"] else be2_sb)
            nc.sync.dma_start(out=out_f[row0:row0 + 128, :], in_=o_sb)

        def unc_filler(c, fi_sb, ft_sb, fc_sb):
            """Generator emitting the uncertainty-head work for one chunk in
            small pieces, used as PE filler between fusion-path stages."""
            for m, (fsb, nk, w1sb) in enumerate(
                    [(fi_sb, KI, w1i_sb), (ft_sb, KT, w1t_sb),
                     (fc_sb, KC, w1c_sb)]):
                hm_ps = ps_tr.tile([128, CH], mybir.dt.float32, tag="aux")
                for k in range(nk):
                    nc.tensor.matmul(hm_ps, w1sb[:, k, :], fsb[:, k, :],
                                     start=(k == 0), stop=(k == nk - 1))
                    yield
                hm_sb = tpool.tile([128, CH], BF, tag="hm")
                nc.scalar.activation(hm_sb, hm_ps, func=AF.Relu,
                                     bias=b1u_sb[:, m:m + 1], scale=1.0)
                sp_ps = ps_tr.tile([1, CH], mybir.dt.float32, tag="aux")
                nc.tensor.matmul(sp_ps, w2u_sb[:, m:m + 1], hm_sb,
                                 start=True, stop=True)
                yield
                sp_st = spool.tile([1, CH], mybir.dt.float32, tag="sp_st")
                nc.vector.tensor_copy(out=sp_st, in_=sp_ps)
                nc.sync.dma_start(out=sp_all[m:m + 1, c * CH:(c + 1) * CH],
                                  in_=sp_st)
                yield

        NT = CH // 128  # batch tiles per chunk
        for c in range(NCH):
            bsl = slice(c * CH, (c + 1) * CH)
            fc_sb = fpool.tile([128, KC, CH], BF, tag="fc")
            nc.sync.dma_start(out=fc_sb, in_=fcT_r[:, :, bsl])
            ft_sb = fpool.tile([128, KT, CH], BF, tag="ft")
            nc.sync.dma_start(out=ft_sb, in_=ftT_r[:, :, bsl])
            fi_sb = fpool.tile([128, KI, CH], BF, tag="fi")
            nc.sync.dma_start(out=fi_sb, in_=fiT_r[:, :, bsl])

            filler = unc_filler(c, fi_sb, ft_sb, fc_sb)
            done = [False]

            def drain(n):
                for _ in range(n):
                    if next(filler, "END") == "END":
                        done[0] = True
                        return

            z = emit_z(fc_sb, ft_sb, 0)
            for i in range(NT):
                t_sb = emit_ln1(*z)
                if i + 1 < NT:
                    z = emit_z(fc_sb, ft_sb, i + 1)
                else:
                    drain(10)
                tr_ps = emit_tr(t_sb)
                drain(3)
                hT = emit_gelu(tr_ps)
                emit_y_out(hT, c * CH + i * 128)
                drain(2)
            while not done[0]:
                drain(1)

        # ---- softplus tail: softplus(x) = ln(1 + exp(x)); exp and ln share
        # one ACT table set. accum_out sums each partition (modality) over
        # all of this core's batch columns.
        e_all = wpool.tile([3, NCH * CH], F32, tag="e_all")
        nc.scalar.activation(e_all, sp_all, func=AF.Exp,
                             bias=b2u_sb[:, 0:1], scale=1.0)
        spv = wpool.tile([3, NCH * CH], F32, tag="spv")
        nc.scalar.activation(spv, e_all, func=AF.Ln, bias=1.0, scale=1.0,
                             accum_out=acc_sb[:, 0:1])
        nc.sync.dma_start(out=out_u[:, :], in_=acc_sb)

    nc.finalize()
    return nc


def _fold_params(params):
    """Fold all pre-LN1 linear algebra into Mc/Mt/zb (float64 on host)."""
    f64 = lambda a: np.asarray(a, dtype=np.float64)
    Wp = {m: f64(params["proj"][m]["w"]) for m in MODS}
    bp = {m: f64(params["proj"][m]["b"]) for m in MODS}

    def fold_pair(src, key):
        a = params["attn"][key]
        wv, bv = f64(a["wv"]), f64(a["bv"])
        wo, bo = f64(a["wo"]), f64(a["bo"])
        G = Wp[src] @ wv @ wo
        g = (bp[src] @ wv + bv) @ wo + bo
        return G, g

    G_img, g_img = fold_pair("clinical", "image_clinical")
    G_txt, g_txt = fold_pair("clinical", "text_clinical")
    G_cli, g_cli = fold_pair("text", "clinical_text")

    fus = params["fus"]
    W1 = f64(fus["w1"])
    b1 = f64(fus["b1"])
    Mc = G_img @ W1[0:FUSION] + G_txt @ W1[FUSION:2 * FUSION]
    Mt = G_cli @ W1[2 * FUSION:3 * FUSION]
    zb = (g_img @ W1[0:FUSION] + g_txt @ W1[FUSION:2 * FUSION]
          + g_cli @ W1[2 * FUSION:3 * FUSION] + b1)
    return Mc, Mt, zb


_CACHE = {}


def kernel(feat_image, feat_text, feat_clinical, params):
    from concourse.bass_utils import run_bass_kernel_spmd

    feat_image = np.asarray(feat_image, dtype=np.float32)
    feat_text = np.asarray(feat_text, dtype=np.float32)
    feat_clinical = np.asarray(feat_clinical, dtype=np.float32)

    Mc, Mt, zb = _fold_params(params)
    fus = params["fus"]
    f32 = lambda a: np.asarray(a, dtype=np.float32)
    W2 = f32(fus["w2"])
    b2f = f32(fus["b2"])
    g1, be1 = f32(fus["g1"]), f32(fus["be1"])
    g2, be2 = f32(fus["g2"]), f32(fus["be2"])
    unc = params["unc"]
    w1 = {m: f32(unc[m]["w1"]) for m in MODS}
    b1u = np.stack([f32(unc[m]["b1"]) for m in MODS], axis=1)  # [128, 3]
    w2u = np.stack([f32(unc[m]["w2"]).reshape(128) for m in MODS], axis=1)
    b2u = np.asarray([f32(unc[m]["b2"]).reshape(()) for m in MODS],
                     dtype=np.float32).reshape(3, 1)

    flags = {
        "zb_triv": bool(np.all(zb == 0.0)),
        "b2f_triv": bool(np.all(b2f == 0.0)),
        "g1_triv": bool(np.all(g1 == 1.0) and np.all(be1 == 0.0)),
        "g2_triv": bool(np.all(g2 == 1.0) and np.all(be2 == 0.0)),
    }

    key = tuple(sorted(flags.items()))
    if key not in _CACHE:
        _CACHE[key] = _build(flags)
    nc = _CACHE[key]

    bf = lambda a: np.ascontiguousarray(a).astype(BF16)
    shared = {
        "Mc": bf(Mc), "Mt": bf(Mt), "W2": bf(W2),
        "w1i": bf(w1["image"]), "w1t": bf(w1["text"]), "w1c": bf(w1["clinical"]),
        "w2u": bf(w2u), "b1u": np.ascontiguousarray(b1u),
        "b2u": np.ascontiguousarray(b2u),
        "zb": bf(zb.reshape(1, -1)), "b2f": bf(b2f.reshape(1, -1)),
        "g1": g1.reshape(1, -1), "be1": be1.reshape(1, -1),
        "g2": g2.reshape(1, -1), "be2": be2.reshape(1, -1),
    }
    in_maps = []
    for c in range(N_CORES):
        sl = slice(c * BS, (c + 1) * BS)
        in_maps.append({
            "fiT": bf(feat_image[sl].T),
            "ftT": bf(feat_text[sl].T),
            "fcT": bf(feat_clinical[sl].T),
            **shared,
        })

    res = run_bass_kernel_spmd(nc, in_maps, core_ids=list(range(N_CORES)))

    fused = np.concatenate(
        [np.asarray(res.results[c]["out_fused"]) for c in range(N_CORES)], axis=0)
    usum = np.zeros(3, dtype=np.float64)
    for c in range(N_CORES):
        usum += np.asarray(res.results[c]["out_unc"], dtype=np.float64).reshape(3)
    uncertainties = (usum / B).astype(np.float32)
    return fused.astype(np.float32), uncertainties


if __name__ == "__main__":
    # lightweight self-check with random data (no reference available)
    rng = np.random.default_rng(0)
    feats = {
        "feat_image": rng.standard_normal((B, D_IMG), dtype=np.float32),
        "feat_text": rng.standard_normal((B, D_TXT), dtype=np.float32),
        "feat_clinical": rng.standard_normal((B, D_CLI), dtype=np.float32),
    }
    print("kernel module loaded; run test.py for the full check")
